# revision 10
# baseline (speedup 1.0000x reference)
"""AdaptiveCLPL loss on 8 TRN2 NeuronCores (Bass/Tile), v3.

loss = mean_b [ psi(avg_cand_b) + sum_head psi(-l)(1-mask) + ts*sum_samp psi(-l)(1-iscand) ]
psi(u) = softplus(-u); psi(-l) = softplus(l) = Ln(Exp(l)+1) (composite; both
funcs forced into the single natural_log_exp_and_others act table).

Decomposition (host does index-driven data movement/layout only; every logit
VALUE is read and combined on device):
  total = sum_b softplus(-avg_b)                       [term1]
        + sum_{head block} softplus(l)                 [bulk DMA + ACT accum]
        + ts * sum_{sampled cols, all rows} softplus(l)
        + sum_cand wcorr * softplus(l_cand),  wcorr = -uniq*(inhead + ts*smult)

Layout (per core, rows = its 256-row batch shard):
- "pref" [128, 2*stot] bf16: the ~2.7k columns the candidate/sampled terms
  touch, pre-transposed on host so batch row b of column s sits at
  (partition b%128, half b//128). Column order [corr | sampled | R0 | R1 |
  pad] with R_g = non-correction candidates of row-group g, so
  - candidate row-sums = one masked mult+reduce per half,
  - correction values = one-hot mult + 2-term reduce, then softplus*wcorr,
  - sampled sums = softplus + accum over the whole sampled block (all rows
    of a sampled column count).
- "lTh" [2000, 256] bf16: head block, bulk-DMA'd as [125, 4096], softplus
  with row-sum accumulation on the ACT engine.
Per-core [128,1] partials are summed on host. No gpsimd/SWDGE anywhere:
plain HWDGE DMAs only (the gather-based variant hit first-execution
SWDGE completion races and a ~9us IRAM library load + ~8.4ns/idx serial
descriptor emission; see kernel_gather.py).
"""

import numpy as np
import ml_dtypes

B, C, K = 2048, 50000, 10
HEAD, S = 2000, 100
TSCALE = float(C - HEAD) / float(S)  # 480.0
NCORES = 8
RB = B // NCORES   # 256
P = 128
HP = 125           # head tile partitions; 2000 = 125*16
HB = HEAD // HP    # 16
BF16 = ml_dtypes.bfloat16

_CACHE = {}


def prep_inputs(logits, candidates, sampled_indices):
    logits = np.asarray(logits)
    candidates = np.asarray(candidates)
    sampled_indices = np.asarray(sampled_indices)
    assert logits.shape == (B, C) and candidates.shape == (B, K)
    srow = (HEAD + sampled_indices.astype(np.int64))      # [S] column ids
    svals, scounts = np.unique(srow, return_counts=True)
    smult_map = dict(zip(svals.tolist(), scounts.tolist()))

    cores = []
    for i in range(NCORES):
        rows = slice(i * RB, (i + 1) * RB)
        cand = candidates[rows].astype(np.int64)
        valid = cand >= 0
        uniq = valid.copy()
        for k in range(1, K):
            dup = (cand[:, :k] == cand[:, k:k + 1]).any(axis=1)
            uniq[:, k] &= ~dup
        cnt = np.maximum(uniq.sum(axis=1), 1).astype(np.float32)
        inhead = cand < HEAD
        mult = np.vectorize(lambda c: smult_map.get(int(c), 0))(cand)
        iscorr = uniq & (inhead | (mult > 0))
        recs = []   # (col, g, p, wcorr, iscorr)
        for b in range(RB):
            for k in range(K):
                if not uniq[b, k]:
                    continue
                recs.append((int(cand[b, k]), b // 128, b % 128,
                             -(float(inhead[b, k])
                               + TSCALE * float(mult[b, k])),
                             bool(iscorr[b, k])))
        cores.append((recs, cnt))

    # shared padded layout across cores (one SPMD graph)
    ng = [0, 0]
    ncorr = 0
    for recs, _ in cores:
        for g in range(2):
            ng[g] = max(ng[g], sum(1 for r in recs
                                   if (not r[4]) and r[1] == g))
        ncorr = max(ncorr, sum(1 for r in recs if r[4]))
    stot = ncorr + S + ng[0] + ng[1]
    stot += (-stot) % 16
    c_lo, c_hi = 0, ncorr
    s_lo, s_hi = ncorr, ncorr + S
    gr = ((s_hi, s_hi + ng[0]), (s_hi + ng[0], s_hi + ng[0] + ng[1]))
    plan = (stot, (c_lo, c_hi), (s_lo, s_hi), gr, ncorr)

    in_maps = []
    for i in range(NCORES):
        recs, cnt = cores[i]
        rows = slice(i * RB, (i + 1) * RB)
        lrows = logits[rows]                              # [256, C] f32
        corr = [r for r in recs if r[4]]
        cols = np.zeros(stot, np.int64)
        mg = [np.zeros((P, ng[g]), np.float32) for g in range(2)]
        for g in range(2):
            sub = [r for r in recs if (not r[4]) and r[1] == g]
            for m, r in enumerate(sub):
                cols[gr[g][0] + m] = r[0]
                mg[g][r[2], m] = 1.0
        ncorr1 = max(ncorr, 1)
        jm_m = np.zeros((P, 2 * ncorr1), np.float32)
        wcpm = np.zeros((P, ncorr1), np.float32)
        for m, (col, g, p, wc, _) in enumerate(corr):
            cols[c_lo + m] = col
            jm_m[p, g * ncorr1 + m] = 1.0
            wcpm[p, m] = wc
        cols[s_lo:s_hi] = srow

        sub = lrows[:, cols].astype(BF16)                 # [256, stot]
        pref = np.concatenate([sub[:128], sub[128:]], axis=1)  # [128, 2*stot]

        rcnt = np.zeros((P, 2), np.float32)
        for b in range(RB):
            rcnt[b % 128, b // 128] = 1.0 / cnt[b]

        maskb = np.concatenate(mg + [jm_m], axis=1).astype(BF16)
        auxf = np.concatenate([rcnt, wcpm], axis=1).astype(np.float32)
        lTh = np.ascontiguousarray(
            lrows[:, :HEAD].T.astype(np.float32)).astype(BF16)

        in_maps.append({
            "pref": np.ascontiguousarray(pref),
            "lTh": lTh,
            "maskb": np.ascontiguousarray(maskb),
            "auxf": np.ascontiguousarray(auxf),
        })
    return in_maps, plan


def _build(plan, enable_asserts=False):
    import os as _os
    import concourse.tile as tile
    from concourse import bacc, mybir

    stot, (c_lo, c_hi), (s_lo, s_hi), gr, ncorr = plan
    ncorr1 = max(ncorr, 1)

    f32 = mybir.dt.float32
    bf16 = mybir.dt.bfloat16
    AF = mybir.ActivationFunctionType
    OP = mybir.AluOpType
    AX = mybir.AxisListType

    nc = bacc.Bacc("TRN2", target_bir_lowering=False, debug=False,
                   enable_asserts=enable_asserts, num_devices=NCORES)

    # one combined exp+ln table -> single ACT_TABLE_LOAD
    from concourse.hw_specs import get_activation_tables
    tabs = get_activation_tables(nc.m.arch)
    if "natural_log_exp_and_others" in tabs:
        for nm, funcs in tabs.items():
            if nm != "natural_log_exp_and_others":
                funcs.discard(AF.Exp)
                funcs.discard(AF.Ln)

    pref = nc.dram_tensor("pref", [P, 2 * stot], bf16,
                          kind="ExternalInput").ap()
    lTh = nc.dram_tensor("lTh", [HEAD, RB], bf16, kind="ExternalInput").ap()
    MW = (gr[0][1] - gr[0][0]) + (gr[1][1] - gr[1][0]) + 2 * ncorr1
    maskb = nc.dram_tensor("maskb", [P, MW], bf16, kind="ExternalInput").ap()
    AW = 2 + ncorr1
    auxf = nc.dram_tensor("auxf", [P, AW], f32, kind="ExternalInput").ap()
    out = nc.dram_tensor("out", [P, 1], f32, kind="ExternalOutput").ap()
    _dbg = _os.environ.get("KDBG", "0") == "1"
    if _dbg:
        dbg = nc.dram_tensor("dbg", [P, 16], f32, kind="ExternalOutput").ap()

    hsrc = lTh.rearrange("(p j) c -> p (j c)", j=HB)      # [125, 4096]

    with tile.TileContext(nc) as tc:
        with tc.tile_pool(name="sb", bufs=1) as sb:
            # --- input DMAs, balanced across both HWDGE rings:
            # sync:   head q0 -> pref half 0 -> head q1
            # scalar: auxf -> maskb -> head q2 -> pref half 1 -> head q3
            ht = sb.tile([HP, HB * RB], bf16)
            hq = HB * RB // 4
            pf = sb.tile([P, 2 * stot], bf16)
            auxf_t = sb.tile([P, AW], f32)
            maskb_t = sb.tile([P, MW], bf16)
            nc.sync.dma_start(out=ht[:, 0 * hq:1 * hq],
                              in_=hsrc[:, 0 * hq:1 * hq])
            nc.scalar.dma_start(out=auxf_t[:, :], in_=auxf[:, :])
            nc.scalar.dma_start(out=maskb_t[:, :], in_=maskb[:, :])
            nc.sync.dma_start(out=pf[:, :stot], in_=pref[:, :stot])
            nc.scalar.dma_start(out=ht[:, 2 * hq:3 * hq],
                                in_=hsrc[:, 2 * hq:3 * hq])
            nc.sync.dma_start(out=ht[:, 1 * hq:2 * hq],
                              in_=hsrc[:, 1 * hq:2 * hq])
            nc.scalar.dma_start(out=pf[:, stot:], in_=pref[:, stot:])
            nc.scalar.dma_start(out=ht[:, 3 * hq:4 * hq],
                                in_=hsrc[:, 3 * hq:4 * hq])

            rcnt_t = auxf_t[:, 0:2]
            wcpm_t = auxf_t[:, 2:2 + ncorr1]
            w0 = gr[0][1] - gr[0][0]
            w1 = gr[1][1] - gr[1][0]
            mg_t = [maskb_t[:, 0:w0], maskb_t[:, w0:w0 + w1]]
            jm_t = maskb_t[:, w0 + w1:w0 + w1 + 2 * ncorr1]

            def pv(g, lo, hi):
                """pref view [128, hi-lo] of half g."""
                return pf[:, :].rearrange(
                    "p (g s) -> p g s", s=stot)[:, g, lo:hi]

            # --- head softplus accum, chunked to overlap with its DMA ---
            hacc2 = sb.tile([HP, 2], f32)
            for hi in range(2):
                nc.scalar.activation(ht[:, hi * 2 * hq:(hi + 1) * 2 * hq],
                                     ht[:, hi * 2 * hq:(hi + 1) * 2 * hq],
                                     AF.Exp)
            for hi in range(2):
                nc.scalar.activation(ht[:, hi * 2 * hq:(hi + 1) * 2 * hq],
                                     ht[:, hi * 2 * hq:(hi + 1) * 2 * hq],
                                     AF.Ln, bias=1.0,
                                     accum_out=hacc2[:, hi:hi + 1])
            hacc = sb.tile([HP, 1], f32)
            nc.vector.tensor_reduce(hacc[:, :], hacc2[:, :], AX.X, OP.add)

            # --- corrections ---
            corr1 = sb.tile([P, 1], f32)
            vc = sb.tile([P, ncorr1], f32)
            if ncorr > 0:
                pc = sb.tile([P, 2 * ncorr1], bf16)
                for g in range(2):
                    nc.vector.tensor_tensor(
                        pc[:, g * ncorr1:g * ncorr1 + ncorr1],
                        pv(g, c_lo, c_lo + ncorr1),
                        jm_t[:, g * ncorr1:g * ncorr1 + ncorr1],
                        op=OP.mult)
                nc.vector.tensor_reduce(
                    vc[:, :],
                    pc[:, :].rearrange("p (g m) -> p m g", g=2),
                    AX.X, OP.add)
                redc = sb.tile([P, 2], f32)
                nc.vector.tensor_reduce(
                    redc[:, :],
                    pc[:, :].rearrange("p (g m) -> p g m", g=2),
                    AX.X, OP.add)
                ce = sb.tile([P, ncorr1], f32)
                nc.scalar.activation(ce[:, :], vc[:, :], AF.Exp)
                spl = sb.tile([P, ncorr1], f32)
                nc.scalar.activation(spl[:, :], ce[:, :], AF.Ln, bias=1.0)
                nc.vector.tensor_tensor(spl[:, :], spl[:, :], wcpm_t,
                                        op=OP.mult)
                nc.vector.tensor_reduce(corr1[:, :], spl[:, :], AX.X, OP.add)
            else:
                nc.vector.memset(corr1[:, :], 0.0)

            # --- sampled: softplus + accum over both halves, all rows ---
            sp = sb.tile([P, 2 * S], bf16)
            for g in range(2):
                nc.scalar.activation(sp[:, g * S:(g + 1) * S],
                                     pv(g, s_lo, s_hi), AF.Exp)
            sacc = sb.tile([P, 1], f32)
            nc.scalar.activation(sp[:, :], sp[:, :], AF.Ln, bias=1.0,
                                 accum_out=sacc[:, :])

            # --- candidate row-sums per half ---
            csum = sb.tile([P, 2], f32)
            prodg = sb.tile([P, max(w0, w1, 1)], bf16)
            for g in range(2):
                if (gr[g][1] - gr[g][0]) == 0:
                    nc.vector.memset(csum[:, g:g + 1], 0.0)
                    continue
                w = gr[g][1] - gr[g][0]
                nc.vector.tensor_tensor(prodg[:, :w],
                                        pv(g, gr[g][0], gr[g][1]),
                                        mg_t[g], op=OP.mult)
                nc.vector.tensor_reduce(csum[:, g:g + 1], prodg[:, :w],
                                        AX.X, OP.add)
            if ncorr > 0:
                nc.vector.tensor_tensor(csum[:, :], csum[:, :], redc[:, :],
                                        op=OP.add)

            # --- term1 ---
            avg = sb.tile([P, 2], f32)
            nc.vector.tensor_tensor(avg[:, :], csum[:, :], rcnt_t,
                                    op=OP.mult)
            ae = sb.tile([P, 2], f32)
            nc.scalar.activation(ae[:, :], avg[:, :], AF.Exp, scale=-1.0)
            t1 = sb.tile([P, 2], f32)
            t1c = sb.tile([P, 1], f32)
            nc.scalar.activation(t1[:, :], ae[:, :], AF.Ln, bias=1.0,
                                 accum_out=t1c[:, :])

            # --- total ---
            total = sb.tile([P, 1], f32)
            nc.vector.tensor_scalar_mul(total[:, :], sacc[:, :], TSCALE)
            nc.vector.tensor_tensor(total[:, :], total[:, :], t1c[:, :],
                                    op=OP.add)
            nc.vector.tensor_tensor(total[:, :], total[:, :], corr1[:, :],
                                    op=OP.add)
            nc.vector.tensor_tensor(total[:HP, :], total[:HP, :],
                                    hacc[:, :], op=OP.add)
            nc.sync.dma_start(out=out[:, :], in_=total[:, :])
            if _dbg:
                dbt = sb.tile([P, 16], f32)
                nc.vector.memset(dbt[:, :], 0.0)
                for col, t, hp in [(0, t1c, P), (1, corr1, P), (2, sacc, P),
                                   (3, hacc, HP)]:
                    nc.vector.tensor_tensor(dbt[:hp, col:col + 1],
                                            dbt[:hp, col:col + 1],
                                            t[:, :], op=OP.add)
                nc.vector.tensor_tensor(dbt[:, 4:6], dbt[:, 4:6],
                                        csum[:, :], op=OP.add)
                nc.vector.tensor_tensor(dbt[:, 6:8], dbt[:, 6:8],
                                        avg[:, :], op=OP.add)
                nc.sync.dma_start(out=dbg[:, :], in_=dbt[:, :])

    nc.compile()
    return nc


def get_graph(plan, enable_asserts=False):
    key = (plan, enable_asserts)
    if key not in _CACHE:
        _CACHE[key] = _build(plan, enable_asserts=enable_asserts)
    return _CACHE[key]


def run(logits, candidates, sampled_indices, trace=False, **kw):
    from concourse.bass_utils import run_bass_kernel_spmd

    in_maps, plan = prep_inputs(logits, candidates, sampled_indices)
    nc = get_graph(plan)
    res = run_bass_kernel_spmd(nc, in_maps, core_ids=list(range(NCORES)),
                               trace=trace, **kw)
    parts = [r["out"].astype(np.float64).sum() for r in res.results]
    loss = np.float32(sum(parts) / B)
    return loss, res


def kernel(logits, candidates, sampled_indices):
    loss, _ = run(logits, candidates, sampled_indices, trace=False)
    return loss


# revision 11
# speedup vs baseline: 1.1284x; 1.1284x over previous
"""AdaptiveCLPL loss on 8 TRN2 NeuronCores (Bass/Tile), v4.

loss = mean_b [ psi(avg_cand_b) + sum_head psi(-l)(1-mask) + ts*sum_samp psi(-l)(1-iscand) ]
psi(u) = softplus(-u); psi(-l) = softplus(l) = Ln(Exp(l)+1) (composite; both
funcs live in the single natural_log_exp_and_others act table -> one load).

Decomposition (host does index-driven data selection/layout only; every
logit VALUE is read, transformed and reduced on device):
  total = sum_b softplus(-avg_b)                       [term1]
        + sum_{head block} softplus(l)                 [bulk DMA + ACT/DVE]
        + ts * sum_{sampled cols, all rows} softplus(l)
        + sum_cand wcorr * softplus(l_cand),  wcorr = -uniq*(inhead + ts*smult)

Per-core layout (rows = its 256-row batch shard; batch row b lives at
partition p=b%128, group g=b//128):
- "pref" [128, 2*GMX + 2*NCP + 2*S] bf16:
    [cand g0 | cand g1]: row (p,g)'s unique non-correction candidate logits,
      padded with 0.0 (adds zero to the row-sum) -> csum = plain row reduce.
    [corr g0 | corr g1]: correction-candidate logits at their row's slot;
      also reduced into csum; softplus(pad=0)=ln2 is cancelled by wcpm=0.
    [samp h0 | samp h1]: the 100 sampled columns for all 256 rows
      (every element needed) -> softplus + accum.
- "lTh" [2000, 256] bf16 head block, viewed [125, 4096]: chunked ACT
  Exp then Ln(+1) with per-chunk row-sum accumulation (overlaps the DMA).
Per-core [128,1] partials are summed on host (no collectives, no gpsimd,
no SWDGE gathers -- see kernel_gather.py for why gathers lose: ~8.4ns/idx
serial descriptor emission + ~9us IRAM library load + first-run races).
"""

import numpy as np
import ml_dtypes

B, C, K = 2048, 50000, 10
HEAD, S = 2000, 100
TSCALE = float(C - HEAD) / float(S)  # 480.0
NCORES = 8
RB = B // NCORES   # 256
P = 128
HP = 125           # head tile partitions; 2000 = 125*16
HB = HEAD // HP    # 16
HW4 = HB * RB // 4  # head tile quarter width (1024)
BF16 = ml_dtypes.bfloat16

_CACHE = {}


def prep_inputs(logits, candidates, sampled_indices):
    logits = np.asarray(logits)
    candidates = np.asarray(candidates)
    sampled_indices = np.asarray(sampled_indices)
    assert logits.shape == (B, C) and candidates.shape == (B, K)
    srow = (HEAD + sampled_indices.astype(np.int64))      # [S] column ids
    svals, scounts = np.unique(srow, return_counts=True)
    smult_map = dict(zip(svals.tolist(), scounts.tolist()))

    cores = []
    for i in range(NCORES):
        rows = slice(i * RB, (i + 1) * RB)
        cand = candidates[rows].astype(np.int64)
        valid = cand >= 0
        uniq = valid.copy()
        for k in range(1, K):
            dup = (cand[:, :k] == cand[:, k:k + 1]).any(axis=1)
            uniq[:, k] &= ~dup
        cnt = np.maximum(uniq.sum(axis=1), 1).astype(np.float32)
        inhead = cand < HEAD
        mult = np.vectorize(lambda c: smult_map.get(int(c), 0))(cand)
        iscorr = uniq & (inhead | (mult > 0))
        plain = [[] for _ in range(RB)]   # candidate col ids per row
        corr = [[] for _ in range(RB)]    # (col, wcorr) per row
        for b in range(RB):
            for k in range(K):
                if not uniq[b, k]:
                    continue
                if iscorr[b, k]:
                    corr[b].append((int(cand[b, k]),
                                    -(float(inhead[b, k])
                                      + TSCALE * float(mult[b, k]))))
                else:
                    plain[b].append(int(cand[b, k]))
        cores.append((plain, corr, cnt))

    gmx = max(max(len(pl) for pl in plain_) or 1
              for plain_, _, _ in cores)
    ncp = max(max(len(co) for co in corr_) or 1
              for _, corr_, _ in cores)
    plan = (gmx, ncp)

    in_maps = []
    for i in range(NCORES):
        plain, corr, cnt = cores[i]
        rows = slice(i * RB, (i + 1) * RB)
        lrows = logits[rows]                              # [256, C] f32

        cv = np.zeros((RB, gmx), np.float32)
        xv = np.zeros((RB, ncp), np.float32)
        wc = np.zeros((RB, ncp), np.float32)
        for b in range(RB):
            for j, col in enumerate(plain[b]):
                cv[b, j] = lrows[b, col]
            for j, (col, w) in enumerate(corr[b]):
                xv[b, j] = lrows[b, col]
                wc[b, j] = w
        sampv = lrows[:, srow]                            # [256, S]

        def fold(a):
            """[256, W] -> [128, 2*W] with (p, g*W + j) = a[g*128+p, j]."""
            return np.concatenate([a[:128], a[128:]], axis=1)

        pref = np.concatenate(
            [fold(cv), fold(xv), fold(sampv)], axis=1).astype(BF16)
        wcpm = fold(wc)                                   # [128, 2*ncp] f32
        rcnt = np.zeros((P, 2), np.float32)
        for b in range(RB):
            rcnt[b % 128, b // 128] = 1.0 / cnt[b]
        auxf = np.concatenate([rcnt, wcpm], axis=1).astype(np.float32)
        lTh = np.ascontiguousarray(
            lrows[:, :HEAD].T.astype(np.float32)).astype(BF16)

        in_maps.append({
            "pref": np.ascontiguousarray(pref),
            "lTh": lTh,
            "auxf": np.ascontiguousarray(auxf),
        })
    return in_maps, plan


def _build(plan, enable_asserts=False):
    import concourse.tile as tile
    from concourse import bacc, mybir

    gmx, ncp = plan
    PW = 2 * gmx + 2 * ncp + 2 * S

    f32 = mybir.dt.float32
    bf16 = mybir.dt.bfloat16
    AF = mybir.ActivationFunctionType
    OP = mybir.AluOpType
    AX = mybir.AxisListType

    nc = bacc.Bacc("TRN2", target_bir_lowering=False, debug=False,
                   enable_asserts=enable_asserts, num_devices=NCORES)

    from concourse.hw_specs import get_activation_tables
    tabs = get_activation_tables(nc.m.arch)
    if "natural_log_exp_and_others" in tabs:
        for nm, funcs in tabs.items():
            if nm != "natural_log_exp_and_others":
                funcs.discard(AF.Exp)
                funcs.discard(AF.Ln)

    pref = nc.dram_tensor("pref", [P, PW], bf16, kind="ExternalInput").ap()
    lTh = nc.dram_tensor("lTh", [HEAD, RB], bf16, kind="ExternalInput").ap()
    AW = 2 + 2 * ncp
    auxf = nc.dram_tensor("auxf", [P, AW], f32, kind="ExternalInput").ap()
    out = nc.dram_tensor("out", [P, 1], f32, kind="ExternalOutput").ap()

    hsrc = lTh.rearrange("(p j) c -> p (j c)", j=HB)      # [125, 4096]

    with tile.TileContext(nc) as tc:
        with tc.tile_pool(name="sb", bufs=1) as sb:
            # --- input DMAs: head quarters first, interleaved on both rings
            ht = sb.tile([HP, HB * RB], bf16)
            pf = sb.tile([P, PW], bf16)
            auxf_t = sb.tile([P, AW], f32)
            nc.sync.dma_start(out=ht[:, 0 * HW4:1 * HW4],
                              in_=hsrc[:, 0 * HW4:1 * HW4])
            nc.scalar.dma_start(out=ht[:, 2 * HW4:3 * HW4],
                                in_=hsrc[:, 2 * HW4:3 * HW4])
            nc.sync.dma_start(out=ht[:, 1 * HW4:2 * HW4],
                              in_=hsrc[:, 1 * HW4:2 * HW4])
            nc.scalar.dma_start(out=pf[:, :], in_=pref[:, :])
            nc.sync.dma_start(out=auxf_t[:, :], in_=auxf[:, :])
            nc.scalar.dma_start(out=ht[:, 3 * HW4:4 * HW4],
                                in_=hsrc[:, 3 * HW4:4 * HW4])

            rcnt_t = auxf_t[:, 0:2]
            wcpm_t = auxf_t[:, 2:2 + 2 * ncp]
            cand_t = pf[:, 0:2 * gmx]
            corr_t = pf[:, 2 * gmx:2 * gmx + 2 * ncp]
            samp_t = pf[:, 2 * gmx + 2 * ncp:PW]

            # --- head: chunked softplus with per-chunk accum ---
            hacc4 = sb.tile([HP, 4], f32)
            for qi in range(4):
                sl = slice(qi * HW4, (qi + 1) * HW4)
                nc.scalar.activation(ht[:, sl], ht[:, sl], AF.Exp)
            for qi in range(4):
                sl = slice(qi * HW4, (qi + 1) * HW4)
                nc.scalar.activation(ht[:, sl], ht[:, sl], AF.Ln, bias=1.0,
                                     accum_out=hacc4[:, qi:qi + 1])
            hacc = sb.tile([HP, 1], f32)
            nc.vector.tensor_reduce(hacc[:, :], hacc4[:, :], AX.X, OP.add)

            # --- sampled: softplus + accum (bf16) ---
            sp = sb.tile([P, 2 * S], bf16)
            nc.scalar.activation(sp[:, :], samp_t, AF.Exp)
            sacc = sb.tile([P, 1], f32)
            nc.scalar.activation(sp[:, :], sp[:, :], AF.Ln, bias=1.0,
                                 accum_out=sacc[:, :])

            # --- corrections: softplus(corr values) dot wcpm ---
            ce = sb.tile([P, 2 * ncp], f32)
            nc.scalar.activation(ce[:, :], corr_t, AF.Exp)
            spl = sb.tile([P, 2 * ncp], f32)
            nc.scalar.activation(spl[:, :], ce[:, :], AF.Ln, bias=1.0)
            nc.vector.tensor_tensor(spl[:, :], spl[:, :], wcpm_t,
                                    op=OP.mult)
            corr1 = sb.tile([P, 1], f32)
            nc.vector.tensor_reduce(corr1[:, :], spl[:, :], AX.X, OP.add)

            # --- candidate row sums -> avg -> term1 ---
            csum = sb.tile([P, 2], f32)
            nc.vector.tensor_reduce(
                csum[:, :], cand_t.rearrange("p (g j) -> p g j", g=2),
                AX.X, OP.add)
            csc = sb.tile([P, 2], f32)
            nc.vector.tensor_reduce(
                csc[:, :], corr_t.rearrange("p (g j) -> p g j", g=2),
                AX.X, OP.add)
            nc.vector.tensor_tensor(csum[:, :], csum[:, :], csc[:, :],
                                    op=OP.add)
            avg = sb.tile([P, 2], f32)
            nc.vector.tensor_tensor(avg[:, :], csum[:, :], rcnt_t,
                                    op=OP.mult)
            ae = sb.tile([P, 2], f32)
            nc.scalar.activation(ae[:, :], avg[:, :], AF.Exp, scale=-1.0)
            t1 = sb.tile([P, 2], f32)
            t1c = sb.tile([P, 1], f32)
            nc.scalar.activation(t1[:, :], ae[:, :], AF.Ln, bias=1.0,
                                 accum_out=t1c[:, :])

            # --- total ---
            total = sb.tile([P, 1], f32)
            nc.vector.tensor_scalar_mul(total[:, :], sacc[:, :], TSCALE)
            nc.vector.tensor_tensor(total[:, :], total[:, :], t1c[:, :],
                                    op=OP.add)
            nc.vector.tensor_tensor(total[:, :], total[:, :], corr1[:, :],
                                    op=OP.add)
            nc.vector.tensor_tensor(total[:HP, :], total[:HP, :],
                                    hacc[:, :], op=OP.add)
            nc.sync.dma_start(out=out[:, :], in_=total[:, :])

    nc.compile()
    return nc


def get_graph(plan, enable_asserts=False):
    key = (plan, enable_asserts)
    if key not in _CACHE:
        _CACHE[key] = _build(plan, enable_asserts=enable_asserts)
    return _CACHE[key]


def run(logits, candidates, sampled_indices, trace=False, **kw):
    from concourse.bass_utils import run_bass_kernel_spmd

    in_maps, plan = prep_inputs(logits, candidates, sampled_indices)
    nc = get_graph(plan)
    res = run_bass_kernel_spmd(nc, in_maps, core_ids=list(range(NCORES)),
                               trace=trace, **kw)
    parts = [r["out"].astype(np.float64).sum() for r in res.results]
    loss = np.float32(sum(parts) / B)
    return loss, res


def kernel(logits, candidates, sampled_indices):
    loss, _ = run(logits, candidates, sampled_indices, trace=False)
    return loss


# revision 12
# speedup vs baseline: 1.2839x; 1.1378x over previous
"""AdaptiveCLPL loss on 8 TRN2 NeuronCores (Bass/Tile), v4.

loss = mean_b [ psi(avg_cand_b) + sum_head psi(-l)(1-mask) + ts*sum_samp psi(-l)(1-iscand) ]
psi(u) = softplus(-u); psi(-l) = softplus(l) = Ln(Exp(l)+1) (composite; both
funcs live in the single natural_log_exp_and_others act table -> one load).

Decomposition (host does index-driven data selection/layout only; every
logit VALUE is read, transformed and reduced on device):
  total = sum_b softplus(-avg_b)                       [term1]
        + sum_{head block} softplus(l)                 [bulk DMA + ACT/DVE]
        + ts * sum_{sampled cols, all rows} softplus(l)
        + sum_cand wcorr * softplus(l_cand),  wcorr = -uniq*(inhead + ts*smult)

Per-core layout (rows = its 256-row batch shard; batch row b lives at
partition p=b%128, group g=b//128):
- "pref" [128, 2*GMX + 2*NCP + 2*S] bf16:
    [cand g0 | cand g1]: row (p,g)'s unique non-correction candidate logits,
      padded with 0.0 (adds zero to the row-sum) -> csum = plain row reduce.
    [corr g0 | corr g1]: correction-candidate logits at their row's slot;
      also reduced into csum; softplus(pad=0)=ln2 is cancelled by wcpm=0.
    [samp h0 | samp h1]: the 100 sampled columns for all 256 rows
      (every element needed) -> softplus + accum.
- "lTh" [2000, 256] bf16 head block, viewed [125, 4096]: chunked ACT
  Exp then Ln(+1) with per-chunk row-sum accumulation (overlaps the DMA).
Per-core [128,1] partials are summed on host (no collectives, no gpsimd,
no SWDGE gathers -- see kernel_gather.py for why gathers lose: ~8.4ns/idx
serial descriptor emission + ~9us IRAM library load + first-run races).
"""

import numpy as np
import ml_dtypes

B, C, K = 2048, 50000, 10
HEAD, S = 2000, 100
TSCALE = float(C - HEAD) / float(S)  # 480.0
NCORES = 8
RB = B // NCORES   # 256
P = 128
HP = 125           # head tile partitions; 2000 = 125*16
HB = HEAD // HP    # 16
HW4 = HB * RB // 4  # head tile quarter width (1024)
BF16 = ml_dtypes.bfloat16

_CACHE = {}


def prep_inputs(logits, candidates, sampled_indices):
    logits = np.asarray(logits)
    candidates = np.asarray(candidates)
    sampled_indices = np.asarray(sampled_indices)
    assert logits.shape == (B, C) and candidates.shape == (B, K)
    srow = (HEAD + sampled_indices.astype(np.int64))      # [S] column ids
    svals, scounts = np.unique(srow, return_counts=True)
    smult_map = dict(zip(svals.tolist(), scounts.tolist()))

    cores = []
    for i in range(NCORES):
        rows = slice(i * RB, (i + 1) * RB)
        cand = candidates[rows].astype(np.int64)
        valid = cand >= 0
        uniq = valid.copy()
        for k in range(1, K):
            dup = (cand[:, :k] == cand[:, k:k + 1]).any(axis=1)
            uniq[:, k] &= ~dup
        cnt = np.maximum(uniq.sum(axis=1), 1).astype(np.float32)
        inhead = cand < HEAD
        mult = np.vectorize(lambda c: smult_map.get(int(c), 0))(cand)
        iscorr = uniq & (inhead | (mult > 0))
        plain = [[] for _ in range(RB)]   # candidate col ids per row
        corr = [[] for _ in range(RB)]    # (col, wcorr) per row
        for b in range(RB):
            for k in range(K):
                if not uniq[b, k]:
                    continue
                if iscorr[b, k]:
                    corr[b].append((int(cand[b, k]),
                                    -(float(inhead[b, k])
                                      + TSCALE * float(mult[b, k]))))
                else:
                    plain[b].append(int(cand[b, k]))
        cores.append((plain, corr, cnt))

    gmx = max(max(len(pl) for pl in plain_) or 1
              for plain_, _, _ in cores)
    ncp = max(max(len(co) for co in corr_) or 1
              for _, corr_, _ in cores)
    plan = (gmx, ncp)

    in_maps = []
    for i in range(NCORES):
        plain, corr, cnt = cores[i]
        rows = slice(i * RB, (i + 1) * RB)
        lrows = logits[rows]                              # [256, C] f32

        cv = np.zeros((RB, gmx), np.float32)
        xv = np.zeros((RB, ncp), np.float32)
        wc = np.zeros((RB, ncp), np.float32)
        for b in range(RB):
            for j, col in enumerate(plain[b]):
                cv[b, j] = lrows[b, col]
            for j, (col, w) in enumerate(corr[b]):
                xv[b, j] = lrows[b, col]
                wc[b, j] = w
        sampv = lrows[:, srow]                            # [256, S]

        def fold(a):
            """[256, W] -> [128, 2*W] with (p, g*W + j) = a[g*128+p, j]."""
            return np.concatenate([a[:128], a[128:]], axis=1)

        rcnt = np.zeros((P, 2), np.float32)
        for b in range(RB):
            rcnt[b % 128, b // 128] = 1.0 / cnt[b]
        pref = np.concatenate(
            [fold(cv), fold(xv), fold(sampv), fold(wc), rcnt],
            axis=1).astype(BF16)
        lTh = np.ascontiguousarray(
            lrows[:, :HEAD].T.astype(np.float32)).astype(BF16)

        in_maps.append({
            "pref": np.ascontiguousarray(pref),
            "lTh": lTh,
        })
    return in_maps, plan


def _build(plan, enable_asserts=False):
    import concourse.tile as tile
    from concourse import bacc, mybir

    gmx, ncp = plan
    PW = 2 * gmx + 2 * ncp + 2 * S + 2 * ncp + 2

    f32 = mybir.dt.float32
    bf16 = mybir.dt.bfloat16
    AF = mybir.ActivationFunctionType
    OP = mybir.AluOpType
    AX = mybir.AxisListType

    nc = bacc.Bacc("TRN2", target_bir_lowering=False, debug=False,
                   enable_asserts=enable_asserts, num_devices=NCORES)

    from concourse.hw_specs import get_activation_tables
    tabs = get_activation_tables(nc.m.arch)
    if "natural_log_exp_and_others" in tabs:
        for nm, funcs in tabs.items():
            if nm != "natural_log_exp_and_others":
                funcs.discard(AF.Exp)
                funcs.discard(AF.Ln)

    pref = nc.dram_tensor("pref", [P, PW], bf16, kind="ExternalInput").ap()
    lTh = nc.dram_tensor("lTh", [HEAD, RB], bf16, kind="ExternalInput").ap()
    out = nc.dram_tensor("out", [P, 1], f32, kind="ExternalOutput").ap()

    hsrc = lTh.rearrange("(p j) c -> p (j c)", j=HB)      # [125, 4096]

    with tile.TileContext(nc) as tc:
        with tc.tile_pool(name="sb", bufs=1) as sb:
            # --- input DMAs: 3 transfers total (fixed ~2us each) ---
            ht = sb.tile([HP, HB * RB], bf16)
            pf = sb.tile([P, PW], bf16)
            half = HB * RB // 2
            nc.scalar.dma_start(out=pf[:, :], in_=pref[:, :])
            nc.sync.dma_start(out=ht[:, :half], in_=hsrc[:, :half])
            nc.scalar.dma_start(out=ht[:, half:], in_=hsrc[:, half:])

            o = 0
            cand_t = pf[:, o:o + 2 * gmx]; o += 2 * gmx
            corr_t = pf[:, o:o + 2 * ncp]; o += 2 * ncp
            samp_t = pf[:, o:o + 2 * S]; o += 2 * S
            wcpm_b = pf[:, o:o + 2 * ncp]; o += 2 * ncp
            rcnt_b = pf[:, o:o + 2]; o += 2
            # f32 working copies of the bf16 weights
            wr = sb.tile([P, 2 * ncp + 2], f32)
            nc.vector.tensor_scalar_mul(wr[:, :], pf[:, o - 2 * ncp - 2:o],
                                        1.0)
            wcpm_t = wr[:, 0:2 * ncp]
            rcnt_t = wr[:, 2 * ncp:2 * ncp + 2]

            # --- sampled: softplus + accum (bf16) ---
            sp = sb.tile([P, 2 * S], bf16)
            nc.scalar.activation(sp[:, :], samp_t, AF.Exp)
            sacc = sb.tile([P, 1], f32)
            nc.scalar.activation(sp[:, :], sp[:, :], AF.Ln, bias=1.0,
                                 accum_out=sacc[:, :])

            # --- corrections: softplus(corr values) dot wcpm ---
            ce = sb.tile([P, 2 * ncp], f32)
            nc.scalar.activation(ce[:, :], corr_t, AF.Exp)
            spl = sb.tile([P, 2 * ncp], f32)
            nc.scalar.activation(spl[:, :], ce[:, :], AF.Ln, bias=1.0)
            nc.vector.tensor_tensor(spl[:, :], spl[:, :], wcpm_t,
                                    op=OP.mult)
            corr1 = sb.tile([P, 1], f32)
            nc.vector.tensor_reduce(corr1[:, :], spl[:, :], AX.X, OP.add)

            # --- candidate row sums -> avg -> term1 ---
            csum = sb.tile([P, 2], f32)
            nc.vector.tensor_reduce(
                csum[:, :], cand_t.rearrange("p (g j) -> p g j", g=2),
                AX.X, OP.add)
            csc = sb.tile([P, 2], f32)
            nc.vector.tensor_reduce(
                csc[:, :], corr_t.rearrange("p (g j) -> p g j", g=2),
                AX.X, OP.add)
            nc.vector.tensor_tensor(csum[:, :], csum[:, :], csc[:, :],
                                    op=OP.add)
            avg = sb.tile([P, 2], f32)
            nc.vector.tensor_tensor(avg[:, :], csum[:, :], rcnt_t,
                                    op=OP.mult)
            ae = sb.tile([P, 2], f32)
            nc.scalar.activation(ae[:, :], avg[:, :], AF.Exp, scale=-1.0)
            t1 = sb.tile([P, 2], f32)
            t1c = sb.tile([P, 1], f32)
            nc.scalar.activation(t1[:, :], ae[:, :], AF.Ln, bias=1.0,
                                 accum_out=t1c[:, :])

            # --- head: softplus per half with accum ---
            hacc2 = sb.tile([HP, 2], f32)
            for hi in range(2):
                sl = slice(hi * 2 * HW4, (hi + 1) * 2 * HW4)
                nc.scalar.activation(ht[:, sl], ht[:, sl], AF.Exp)
            for hi in range(2):
                sl = slice(hi * 2 * HW4, (hi + 1) * 2 * HW4)
                nc.scalar.activation(ht[:, sl], ht[:, sl], AF.Ln, bias=1.0,
                                     accum_out=hacc2[:, hi:hi + 1])
            hacc = sb.tile([HP, 1], f32)
            nc.vector.tensor_reduce(hacc[:, :], hacc2[:, :], AX.X, OP.add)

            # --- total ---
            total = sb.tile([P, 1], f32)
            nc.vector.tensor_scalar_mul(total[:, :], sacc[:, :], TSCALE)
            nc.vector.tensor_tensor(total[:, :], total[:, :], t1c[:, :],
                                    op=OP.add)
            nc.vector.tensor_tensor(total[:, :], total[:, :], corr1[:, :],
                                    op=OP.add)
            nc.vector.tensor_tensor(total[:HP, :], total[:HP, :],
                                    hacc[:, :], op=OP.add)
            nc.sync.dma_start(out=out[:, :], in_=total[:, :])

    nc.compile()
    return nc


def get_graph(plan, enable_asserts=False):
    key = (plan, enable_asserts)
    if key not in _CACHE:
        _CACHE[key] = _build(plan, enable_asserts=enable_asserts)
    return _CACHE[key]


def run(logits, candidates, sampled_indices, trace=False, **kw):
    from concourse.bass_utils import run_bass_kernel_spmd

    in_maps, plan = prep_inputs(logits, candidates, sampled_indices)
    nc = get_graph(plan)
    res = run_bass_kernel_spmd(nc, in_maps, core_ids=list(range(NCORES)),
                               trace=trace, **kw)
    parts = [r["out"].astype(np.float64).sum() for r in res.results]
    loss = np.float32(sum(parts) / B)
    return loss, res


def kernel(logits, candidates, sampled_indices):
    loss, _ = run(logits, candidates, sampled_indices, trace=False)
    return loss
